# revision 15
# baseline (speedup 1.0000x reference)
"""Batched ADMM-QP (nn_BackwardStep) Trainium2 kernel, v3.

Math (per batch element n, matching the jax reference):
    M = Q + I + A^T A             (A = [A_ineq; A_eq], rho = alpha = 1)
    Y ~= M^-1                     (deg-4 minimax seed on [1.1,7.7], then one
                                   fp32-split Newton polish: X += X1 (I - M X))
    G = A Y A^T (640x640, hi/lo bf16 split), e = A Y q2, y0 = Y q2, E = A Y
    Over-relaxed ADMM (alpha_r = 1.6) run to convergence instead of the
    reference's 100 plain steps (same fixed point; the reference is ~1.6e-3
    from its limit at step 100, far inside the 2e-2 gate):
        t = G s;  v = a t + (1-a) z + w - a e = a t + c
        z' = min(v, u) (ineq rows; eq rows stay b_eq);  s' = 2 z' - v
        c' = v - a (z' + e)
    30 "fast" rounds use a single-pass bf16 matvec (G1 s1); 12 "clean"
    rounds use the exact 3-pass split (G1 s1 + G2 s1 + G1 s2) so the fast
    rounds' rounding noise decays geometrically before the final solve.
    xs = -y0 + E1^T (s1 + s2)

Sharding: batch dim 64 -> 8 cores x 8 elements, zero cross-core traffic.
"""

import numpy as np

import concourse.bass as bass
import concourse.mybir as mybir
import concourse.tile as tile
from concourse import bacc
from concourse import bass_utils

F32 = mybir.dt.float32
BF16 = mybir.dt.bfloat16
ALU = mybir.AluOpType

D = 512          # primal dim
MI = 512         # ineq constraints
ME = 128         # eq constraints
M = MI + ME      # 640
NC = D // 128    # 4 d-chunks
MC = M // 128    # 5 m-chunks
EPC = 8          # batch elems per core
ALPHA = 1.6      # ADMM over-relaxation
N_FAST = 30      # 1-pass bf16 matvec rounds
N_CLEAN = 12     # exact split matvec rounds

# Degree-4 minimax poly for 1/t on [1.1, 7.7] (residual 0.0375); applied
# via Horner in M^2:  X0 = (P0 I + P1 M + P2 M2) + M2 @ (P3 M + P4 M2)
P0c = 1.7168134148393248
P1c = -1.0298713680464564
P2c = 0.27577563635807445
P3c = -0.03370825196126197
P4c = 0.0015321932709664529


def build_program():
    nc = bacc.Bacc("TRN2", target_bir_lowering=False)

    Q8 = nc.declare_dram_parameter("Q8", [EPC, D, D], F32, isOutput=False)
    q8 = nc.declare_dram_parameter("q8", [EPC, D], F32, isOutput=False)
    Ai8 = nc.declare_dram_parameter("Ai8", [EPC, MI, D], F32, isOutput=False)
    bi8 = nc.declare_dram_parameter("bi8", [EPC, MI], F32, isOutput=False)
    Ae8 = nc.declare_dram_parameter("Ae8", [EPC, ME, D], F32, isOutput=False)
    be8 = nc.declare_dram_parameter("be8", [EPC, ME], F32, isOutput=False)
    x8 = nc.declare_dram_parameter("x8", [EPC, D], F32, isOutput=False)
    identD = nc.declare_dram_parameter("identD", [128, 128], F32, isOutput=False)
    xs8 = nc.declare_dram_parameter("xs8", [EPC, D], F32, isOutput=True)

    # DRAM scratch: E1 (final solve) and G2 (clean rounds), reloaded later
    E1d = nc.dram_tensor("E1d", [EPC, 128, MC * D], BF16)
    G2d = nc.dram_tensor("G2d", [EPC, 128, MC * M], BF16)

    with tile.TileContext(nc) as tc:
        with tc.tile_pool(name="pers", bufs=1) as P0:
            ident = P0.tile([128, 128], F32)
            nc.sync.dma_start(ident[:], identD[:])
            identb = P0.tile([128, 128], BF16)
            nc.vector.tensor_copy(identb[:], ident[:])

            # persistent state (all [128, chunk, elem] layouts)
            G1t = P0.tile([128, EPC, MC, M], BF16)
            tcol = P0.tile([128, MC, EPC], F32)
            vcol = P0.tile([128, MC, EPC], F32)
            zcol = P0.tile([128, MC, EPC], F32)
            ccol = P0.tile([128, MC, EPC], F32)
            ecol = P0.tile([128, MC, EPC], F32)
            zetmp = P0.tile([128, MC, EPC], F32)
            sfc = P0.tile([128, MC, EPC], F32)
            uineq = P0.tile([128, 4, EPC], F32)
            s1c = P0.tile([128, MC, EPC], BF16)
            s2c = P0.tile([128, MC, EPC], BF16)
            trowAa = P0.tile([128, 384], F32)
            trowAb = P0.tile([128, 256], F32)
            trowBa = P0.tile([128, 384], F32)
            trowBb = P0.tile([128, 256], F32)
            y0colP = P0.tile([128, NC, EPC], F32)

            # ---------------- per-element precompute ----------------
            with (
                tc.tile_pool(name="pre", bufs=1) as P1,
                tc.tile_pool(name="prep", bufs=1, space="PSUM") as PSA,
            ):
                def split_chunk(dst1, dst2, src_f32):
                    """dst1/dst2 (bf16 APs) = hi/lo split of src_f32 AP."""
                    nc.scalar.copy(dst1, src_f32)
                    nc.vector.tensor_sub(dst2, src_f32, dst1)

                def split_chunk_g(dst1, dst2, src_f32):
                    """split with the lo-sub on gpsimd (SBUF sources only)."""
                    nc.scalar.copy(dst1, src_f32)
                    nc.gpsimd.tensor_sub(dst2, src_f32, dst1)

                for e in range(EPC):
                    # -- load A ([m-part, mchunk, d] layout) and split
                    A5f = P1.tile([128, MC, D], F32, tag="A5f")
                    nc.sync.dma_start(
                        A5f[:, 0:4, :],
                        Ai8[e].rearrange("(c p) d -> p c d", p=128))
                    nc.sync.dma_start(A5f[:, 4, :], Ae8[e])
                    A5b1 = P1.tile([128, MC, D], BF16, tag="A5b1", bufs=2)
                    A5b2 = P1.tile([128, MC, D], BF16, tag="A5b2")
                    for j in range(MC):
                        split_chunk_g(A5b1[:, j, :], A5b2[:, j, :], A5f[:, j, :])

                    # -- M = (A1+A2)^T A1 + Q + I  (f32 Mf kept; M1b bf16)
                    Mf = P1.tile([128, NC, D], F32, tag="Mf")
                    M1b = P1.tile([128, NC, D], BF16, tag="M1b", bufs=2)
                    for i in range(NC):
                        ps = PSA.tile([128, D], F32, tag="acc", bufs=2)
                        for pi, la in enumerate((A5b1, A5b2)):
                            for j in range(MC):
                                nc.tensor.matmul(
                                    ps[:], la[:, j, 128 * i:128 * (i + 1)],
                                    A5b1[:, j, :],
                                    start=(pi == 0 and j == 0),
                                    stop=(pi == 1 and j == MC - 1))
                        qblk = P1.tile([128, D], F32, tag="qblk", bufs=2)
                        nc.sync.dma_start(
                            qblk[:],
                            Q8[e].rearrange("(c p) d -> p c d", p=128)[:, i, :])
                        nc.vector.tensor_add(Mf[:, i, :], ps[:], qblk[:])
                        nc.vector.tensor_add(
                            Mf[:, i, 128 * i:128 * (i + 1)],
                            Mf[:, i, 128 * i:128 * (i + 1)], ident[:])
                        nc.vector.tensor_copy(M1b[:, i, :], Mf[:, i, :])

                    # -- M2 = M1 @ M1 ; Q2b = bf16(P3 Mf + P4 M2)
                    M2f = P1.tile([128, NC, D], F32, tag="M2f")
                    M2b = P1.tile([128, NC, D], BF16, tag="M2b")
                    Q2b = P1.tile([128, NC, D], BF16, tag="Q2b")
                    for i in range(NC):
                        ps = PSA.tile([128, D], F32, tag="acc", bufs=2)
                        for k in range(NC):
                            nc.tensor.matmul(
                                ps[:], M1b[:, k, 128 * i:128 * (i + 1)],
                                M1b[:, k, :],
                                start=(k == 0), stop=(k == NC - 1))
                        nc.vector.tensor_copy(M2f[:, i, :], ps[:])
                        t3 = P1.tile([128, D], F32, tag="t3", bufs=2)
                        nc.vector.tensor_scalar_mul(t3[:], Mf[:, i, :], P3c)
                        nc.vector.scalar_tensor_tensor(
                            Q2b[:, i, :], ps[:], P4c, t3[:],
                            op0=ALU.mult, op1=ALU.add)
                        nc.scalar.copy(M2b[:, i, :], ps[:])

                    # -- X0 = P0 I + P1 Mf + P2 M2f + M2b @ Q2b
                    X0f = P1.tile([128, NC, D], F32, tag="X0f")
                    X1p = P1.tile([128, NC, D], BF16, tag="X1p")
                    X2p = P1.tile([128, NC, D], BF16, tag="X2p")
                    for i in range(NC):
                        ps = PSA.tile([128, D], F32, tag="acc", bufs=2)
                        for k in range(NC):
                            nc.tensor.matmul(
                                ps[:], M2b[:, k, 128 * i:128 * (i + 1)],
                                Q2b[:, k, :],
                                start=(k == 0), stop=(k == NC - 1))
                        nc.vector.scalar_tensor_tensor(
                            X0f[:, i, :], Mf[:, i, :], P1c, ps[:],
                            op0=ALU.mult, op1=ALU.add)
                        nc.vector.scalar_tensor_tensor(
                            X0f[:, i, :], M2f[:, i, :], P2c, X0f[:, i, :],
                            op0=ALU.mult, op1=ALU.add)
                        nc.vector.scalar_tensor_tensor(
                            X0f[:, i, 128 * i:128 * (i + 1)], ident[:], P0c,
                            X0f[:, i, 128 * i:128 * (i + 1)],
                            op0=ALU.mult, op1=ALU.add)
                        split_chunk_g(X1p[:, i, :], X2p[:, i, :], X0f[:, i, :])

                    # -- fp32-split Newton polish: R = I - M1(X1p + X2p),
                    #    X = X0 + X1p R
                    Rm = P1.tile([128, NC, D], BF16, tag="Rm")
                    for i in range(NC):
                        ps = PSA.tile([128, D], F32, tag="acc", bufs=2)
                        for pi, ra in enumerate((X1p, X2p)):
                            for k in range(NC):
                                nc.tensor.matmul(
                                    ps[:], M1b[:, k, 128 * i:128 * (i + 1)],
                                    ra[:, k, :],
                                    start=(pi == 0 and k == 0),
                                    stop=(pi == 1 and k == NC - 1))
                        nc.vector.tensor_scalar_mul(Rm[:, i, :], ps[:], -1.0)
                        rfd = P1.tile([128, 128], F32, tag="rfd", bufs=1)
                        nc.vector.tensor_sub(
                            rfd[:], ident[:], ps[:, 128 * i:128 * (i + 1)])
                        nc.vector.tensor_copy(
                            Rm[:, i, 128 * i:128 * (i + 1)], rfd[:])
                    Xf = P1.tile([128, NC, D], F32, tag="Xf")
                    X1 = P1.tile([128, NC, D], BF16, tag="X1", bufs=2)
                    for i in range(NC):
                        ps = PSA.tile([128, D], F32, tag="acc", bufs=2)
                        for k in range(NC):
                            nc.tensor.matmul(
                                ps[:], X1p[:, k, 128 * i:128 * (i + 1)],
                                Rm[:, k, :],
                                start=(k == 0), stop=(k == NC - 1))
                        nc.vector.tensor_add(Xf[:, i, :], X0f[:, i, :], ps[:])
                        nc.vector.tensor_copy(X1[:, i, :], Xf[:, i, :])

                    # -- AT1 = A5b1^T ([d-part, dchunk, m]) via PE transposes
                    AT1 = P1.tile([128, NC, M], BF16, tag="AT1", bufs=2)
                    for j in range(MC):
                        for k in range(NC):
                            tp = PSA.tile([128, 128], BF16, tag="tp", bufs=2)
                            nc.tensor.transpose(
                                tp[:], A5b1[:, j, 128 * k:128 * (k + 1)],
                                identb[:])
                            if k % 2 == 0:
                                nc.vector.tensor_copy(
                                    AT1[:, k, 128 * j:128 * (j + 1)], tp[:])
                            else:
                                nc.scalar.copy(
                                    AT1[:, k, 128 * j:128 * (j + 1)], tp[:])

                    # -- q2 = q - x (column form [128, NC]) and splits
                    qc = P1.tile([128, NC], F32, tag="qc", bufs=2)
                    xc = P1.tile([128, NC], F32, tag="xc", bufs=2)
                    nc.sync.dma_start(qc[:], q8[e].rearrange("(c p) -> p c", p=128))
                    nc.sync.dma_start(xc[:], x8[e].rearrange("(c p) -> p c", p=128))
                    q2c = P1.tile([128, NC], F32, tag="q2c", bufs=2)
                    nc.vector.tensor_sub(q2c[:], qc[:], xc[:])
                    q21 = P1.tile([128, NC], BF16, tag="q21", bufs=2)
                    q22 = P1.tile([128, NC], BF16, tag="q22", bufs=2)
                    split_chunk_g(q21[:], q22[:], q2c[:])

                    # -- clip bounds: ineq into uineq, eq into zcol chunk 4
                    nc.sync.dma_start(
                        uineq[:, :, e], bi8[e].rearrange("(c p) -> p c", p=128))
                    nc.sync.dma_start(zcol[:, 4, e:e + 1],
                                      be8[e:e + 1].rearrange('o p -> p o'))

                    # -- y0 row = (q21+q22)^T X1, then column
                    psr = PSA.tile([1, D], F32, tag="row", bufs=1)
                    for pi, la in enumerate((q21, q22)):
                        for k in range(NC):
                            nc.tensor.matmul(
                                psr[:], la[:, k:k + 1], X1[:, k, :],
                                start=(pi == 0 and k == 0),
                                stop=(pi == 1 and k == NC - 1))
                    y0rt = P1.tile([1, M], F32, tag="rowst", bufs=1)
                    nc.vector.tensor_copy(y0rt[:, 0:D], psr[:])
                    for k in range(NC):
                        tp = PSA.tile([128, 128], F32, tag="tpf", bufs=1)
                        nc.tensor.transpose(
                            tp[:, 0:1],
                            y0rt[:, 128 * k:128 * (k + 1)],
                            ident[0:1, 0:1])
                        nc.vector.tensor_copy(y0colP[:, k, e:e + 1], tp[:, 0:1])
                    y01 = P1.tile([128, NC], BF16, tag="y01", bufs=2)
                    y02 = P1.tile([128, NC], BF16, tag="y02", bufs=2)
                    split_chunk_g(y01[:], y02[:], y0colP[:, :, e])

                    # -- e row = (y01+y02)^T AT1 (spans 384 + 256) -> ecol
                    erow = P1.tile([1, M], F32, tag="rowst", bufs=1)
                    for lo, hi in ((0, 384), (384, 640)):
                        pse = PSA.tile([1, D], F32, tag="row", bufs=1)
                        for pi, la in enumerate((y01, y02)):
                            for k in range(NC):
                                nc.tensor.matmul(
                                    pse[:, 0:hi - lo], la[:, k:k + 1],
                                    AT1[:, k, lo:hi],
                                    start=(pi == 0 and k == 0),
                                    stop=(pi == 1 and k == NC - 1))
                        nc.vector.tensor_copy(erow[:, lo:hi], pse[:, 0:hi - lo])
                    for j in range(MC):
                        tp = PSA.tile([128, 128], F32, tag="tpf", bufs=1)
                        nc.tensor.transpose(
                            tp[:, 0:1], erow[:, 128 * j:128 * (j + 1)],
                            ident[0:1, 0:1])
                        nc.vector.tensor_copy(ecol[:, j, e:e + 1], tp[:, 0:1])

                    # -- Dm = X1 @ AT1 ([d-part, dchunk, m]), D1 bf16 (1-pass)
                    D1 = P1.tile([128, NC, M], BF16, tag="D1", bufs=2)
                    for i in range(NC):
                        for lo, hi in ((0, 384), (384, 640)):
                            ps = PSA.tile([128, 384], F32, tag="accm", bufs=2)
                            for k in range(NC):
                                nc.tensor.matmul(
                                    ps[:, 0:hi - lo],
                                    X1[:, k, 128 * i:128 * (i + 1)],
                                    AT1[:, k, lo:hi],
                                    start=(k == 0), stop=(k == NC - 1))
                            if lo == 0:
                                nc.scalar.copy(D1[:, i, lo:hi], ps[:, 0:hi - lo])
                            else:
                                nc.vector.tensor_copy(
                                    D1[:, i, lo:hi], ps[:, 0:hi - lo])

                    # -- E1 = D1^T via bf16 PE transposes, spilled to DRAM
                    E1 = P1.tile([128, MC, D], BF16, tag="E1", bufs=2)
                    for j in range(MC):
                        for k in range(NC):
                            tp = PSA.tile([128, 128], BF16, tag="tp", bufs=2)
                            nc.tensor.transpose(
                                tp[:], D1[:, k, 128 * j:128 * (j + 1)],
                                identb[:])
                            if k % 2 == 0:
                                nc.vector.tensor_copy(
                                    E1[:, j, 128 * k:128 * (k + 1)], tp[:])
                            else:
                                nc.scalar.copy(
                                    E1[:, j, 128 * k:128 * (k + 1)], tp[:])
                    nc.sync.dma_start(E1d[e], E1[:].rearrange("p c d -> p (c d)"))

                    # -- G = AT1^T @ D1, split G1 (SBUF) / G2 (DRAM spill)
                    G2e = P1.tile([128, MC, M], BF16, tag="G2e", bufs=1)
                    for j in range(MC):
                        for lo, hi in ((0, 384), (384, 640)):
                            ps = PSA.tile([128, 384], F32, tag="accm", bufs=2)
                            for k in range(NC):
                                nc.tensor.matmul(
                                    ps[:, 0:hi - lo],
                                    AT1[:, k, 128 * j:128 * (j + 1)],
                                    D1[:, k, lo:hi],
                                    start=(k == 0), stop=(k == NC - 1))
                            split_chunk(G1t[:, e, j, lo:hi],
                                        G2e[:, j, lo:hi], ps[:, 0:hi - lo])
                    nc.sync.dma_start(G2d[e], G2e[:].rearrange("p c d -> p (c d)"))

            # ---------------- ADMM iterations ----------------
            def tail(w, boot=False, with_s2=True):
                S = slice(4 * w, 4 * w + 4)
                if boot:
                    nc.vector.tensor_scalar_mul(
                        vcol[:, :, S], ecol[:, :, S], -1.0)
                else:
                    nc.vector.scalar_tensor_tensor(
                        vcol[:, :, S], tcol[:, :, S], ALPHA, ccol[:, :, S],
                        op0=ALU.mult, op1=ALU.add)
                nc.vector.tensor_tensor(
                    zcol[:, 0:4, S], vcol[:, 0:4, S], uineq[:, :, S],
                    op=ALU.min)
                nc.vector.scalar_tensor_tensor(
                    s1c[:, :, S], zcol[:, :, S], 2.0, vcol[:, :, S],
                    op0=ALU.mult, op1=ALU.subtract)
                if with_s2:
                    nc.vector.scalar_tensor_tensor(
                        sfc[:, :, S], zcol[:, :, S], 2.0, vcol[:, :, S],
                        op0=ALU.mult, op1=ALU.subtract)
                    nc.gpsimd.tensor_sub(
                        s2c[:, :, S], sfc[:, :, S], s1c[:, :, S])
                nc.gpsimd.tensor_add(
                    zetmp[:, :, S], zcol[:, :, S], ecol[:, :, S])
                nc.vector.scalar_tensor_tensor(
                    ccol[:, :, S], zetmp[:, :, S], -ALPHA, vcol[:, :, S],
                    op0=ALU.mult, op1=ALU.add)

            with tc.tile_pool(name="adm", bufs=1) as PA:
              with tc.tile_pool(name="itp", bufs=1, space="PSUM") as PSI:
                G2sb = PA.tile([128, EPC, MC, M], BF16)
                for e in range(EPC):
                    nc.sync.dma_start(
                        G2sb[:, e].rearrange("p c d -> p (c d)"), G2d[e])
                E1all = PA.tile([128, EPC, MC, D], BF16)
                for e in range(EPC):
                    nc.sync.dma_start(
                        E1all[:, e].rearrange("p c d -> p (c d)"), E1d[e])

                tail(0, boot=True)
                tail(1, boot=True)

                def mm_wave(wave, k, clean):
                    pA = PSI.tile([128, 384], F32, tag="wvA", bufs=2,
                                  name=f"pA_{k}_{wave}")
                    pB = PSI.tile([128, 256], F32, tag="wvB", bufs=2,
                                  name=f"pB_{k}_{wave}")
                    passes = (((G1t, s1c), (G2sb, s1c), (G1t, s2c))
                              if clean else ((G1t, s1c),))
                    np_ = len(passes)
                    for pi, (Gt, st) in enumerate(passes):
                        for j in range(MC):
                            first = pi == 0 and j == 0
                            last = pi == np_ - 1 and j == MC - 1
                            for eo in range(4):
                                e = 4 * wave + eo
                                ga = (Gt[:, e, j, 0:384] if Gt is G1t
                                      else Gt[:, e, j, 0:384])
                                gb = (Gt[:, e, j, 384:640] if Gt is G1t
                                      else Gt[:, e, j, 384:640])
                                nc.tensor.matmul(
                                    pA[32 * eo:32 * eo + 1, :],
                                    st[:, j, e:e + 1], ga,
                                    start=first, stop=last,
                                    tile_position=(0, 32 * eo))
                                nc.tensor.matmul(
                                    pB[32 * eo:32 * eo + 1, :],
                                    st[:, j, e:e + 1], gb,
                                    start=first, stop=last,
                                    tile_position=(0, 32 * eo))
                    return pA, pB

                def post_wave(wave, pA, pB, k, with_s2=True):
                    ta = trowAa if wave == 0 else trowBa
                    tb = trowAb if wave == 0 else trowBb
                    # full-tile copies: rows 32*eo carry t, the rest is
                    # garbage the column extraction never reads; 128 lanes
                    # make these ~8x cheaper than per-row copies
                    nc.vector.tensor_copy(ta[:], pA[:])
                    nc.scalar.copy(tb[:], pB[:])
                    T2a = PSI.tile([128, 3, 128], F32, tag="T2a", bufs=2,
                                   name=f"t2a_{k}_{wave}")
                    T2b = PSI.tile([128, 2, 128], F32, tag="T2b", bufs=2,
                                   name=f"t2b_{k}_{wave}")
                    for j in range(MC):
                        src_ap = (ta[:, 128 * j:128 * (j + 1)] if j < 3
                                  else tb[:, 128 * (j - 3):128 * (j - 2)])
                        dst = T2a[:, j, :] if j < 3 else T2b[:, j - 3, :]
                        nc.tensor.transpose(dst, src_ap, ident[:])
                    S4 = slice(4 * wave, 4 * wave + 4)
                    nc.vector.tensor_copy(
                        tcol[:, 0:3, S4],
                        T2a.rearrange("p c (a b) -> p c a b", b=32)[:, :, :, 0])
                    nc.scalar.copy(
                        tcol[:, 3:5, S4],
                        T2b.rearrange("p c (a b) -> p c a b", b=32)[:, :, :, 0])
                    tail(wave, with_s2=with_s2)

                for k in range(N_FAST + N_CLEAN):
                    clean = k >= N_FAST
                    with_s2 = k >= N_FAST - 1
                    pA0, pB0 = mm_wave(0, k, clean)
                    pA1, pB1 = mm_wave(1, k, clean)
                    post_wave(0, pA0, pB0, k, with_s2=with_s2)
                    post_wave(1, pA1, pB1, k, with_s2=with_s2)

              # ------------- final solve: xs = E1^T (s1+s2) - y0 ----------
              with (
                tc.tile_pool(name="fin", bufs=1) as PF,
                tc.tile_pool(name="finp", bufs=1, space="PSUM") as PSF,
              ):
                for e in range(EPC):
                    ps = PSF.tile([1, D], F32, tag="frow", bufs=2)
                    for pi, st in enumerate((s1c, s2c)):
                        for j in range(MC):
                            nc.tensor.matmul(
                                ps[:], st[:, j, e:e + 1], E1all[:, e, j, :],
                                start=(pi == 0 and j == 0),
                                stop=(pi == 1 and j == MC - 1))
                    yrt = PF.tile([1, D], F32, tag="yrt", bufs=2)
                    for k in range(NC):
                        tpf = PSF.tile([128, 128], F32, tag="tpf", bufs=2)
                        nc.tensor.transpose(
                            tpf[0:1, :], y0colP[:, k, e:e + 1], ident[:])
                        nc.vector.tensor_copy(
                            yrt[:, 128 * k:128 * (k + 1)], tpf[0:1, :])
                    xr = PF.tile([1, D], F32, tag="xr", bufs=2)
                    nc.vector.scalar_tensor_tensor(
                        xr[:], ps[:], 1.0, yrt[:],
                        op0=ALU.mult, op1=ALU.subtract)
                    nc.sync.dma_start(xs8[e:e + 1, :], xr[:])

    nc.finalize()
    return nc


_CACHED = {}


def _get_program():
    if "nc" not in _CACHED:
        _CACHED["nc"] = build_program()
    return _CACHED["nc"]


def run(inputs, trace=False, trace_cores=None):
    nc = _get_program()
    Q = np.ascontiguousarray(inputs["Q"], dtype=np.float32)
    q = np.ascontiguousarray(inputs["q"], dtype=np.float32)[..., 0]
    Ai = np.ascontiguousarray(inputs["A_ineq"], dtype=np.float32)
    bi = np.ascontiguousarray(inputs["b_ineq"], dtype=np.float32)[..., 0]
    Ae = np.ascontiguousarray(inputs["A_eq"], dtype=np.float32)
    be = np.ascontiguousarray(inputs["b_eq"], dtype=np.float32)[..., 0]
    x = np.ascontiguousarray(inputs["x"], dtype=np.float32)[..., 0]
    ident = np.eye(128, dtype=np.float32)

    in_maps = []
    for c in range(8):
        s = slice(EPC * c, EPC * (c + 1))
        in_maps.append({
            "Q8": Q[s], "q8": q[s], "Ai8": Ai[s], "bi8": bi[s],
            "Ae8": Ae[s], "be8": be[s], "x8": x[s], "identD": ident,
        })
    res = bass_utils.run_bass_kernel_spmd(
        nc, in_maps, list(range(8)), trace=trace,
        trace_cores=trace_cores)
    out = np.concatenate([res.results[c]["xs8"] for c in range(8)], axis=0)
    return out[..., None].astype(np.float32), res


def kernel(**inputs):
    out, _ = run(inputs, trace=False)
    return out


# revision 17
# speedup vs baseline: 1.0529x; 1.0529x over previous
"""Batched ADMM-QP (nn_BackwardStep) Trainium2 kernel, v3.

Math (per batch element n, matching the jax reference):
    M = Q + I + A^T A             (A = [A_ineq; A_eq], rho = alpha = 1)
    Y ~= M^-1                     (deg-4 minimax seed on [1.1,7.7], then one
                                   fp32-split Newton polish: X += X1 (I - M X))
    G = A Y A^T (640x640, hi/lo bf16 split), e = A Y q2, y0 = Y q2, E = A Y
    Over-relaxed ADMM (alpha_r = 1.6) run to convergence instead of the
    reference's 100 plain steps (same fixed point; the reference is ~1.6e-3
    from its limit at step 100, far inside the 2e-2 gate):
        t = G s;  v = a t + (1-a) z + w - a e = a t + c
        z' = min(v, u) (ineq rows; eq rows stay b_eq);  s' = 2 z' - v
        c' = v - a (z' + e)
    30 "fast" rounds use a single-pass bf16 matvec (G1 s1); 12 "clean"
    rounds use the exact 3-pass split (G1 s1 + G2 s1 + G1 s2) so the fast
    rounds' rounding noise decays geometrically before the final solve.
    xs = -y0 + E1^T (s1 + s2)

Sharding: batch dim 64 -> 8 cores x 8 elements, zero cross-core traffic.
"""

import numpy as np

import concourse.bass as bass
import concourse.mybir as mybir
import concourse.tile as tile
from concourse import bacc
from concourse import bass_utils

F32 = mybir.dt.float32
BF16 = mybir.dt.bfloat16
ALU = mybir.AluOpType

D = 512          # primal dim
MI = 512         # ineq constraints
ME = 128         # eq constraints
M = MI + ME      # 640
NC = D // 128    # 4 d-chunks
MC = M // 128    # 5 m-chunks
EPC = 8          # batch elems per core
ALPHA = 1.7      # ADMM over-relaxation
N_FAST = 28      # 1-pass bf16 matvec rounds
N_CLEAN = 12     # exact split matvec rounds

# Degree-4 minimax poly for 1/t on [1.1, 7.7] (residual 0.0375); applied
# via Horner in M^2:  X0 = (P0 I + P1 M + P2 M2) + M2 @ (P3 M + P4 M2)
P0c = 1.7168134148393248
P1c = -1.0298713680464564
P2c = 0.27577563635807445
P3c = -0.03370825196126197
P4c = 0.0015321932709664529


def build_program():
    nc = bacc.Bacc("TRN2", target_bir_lowering=False)

    Q8 = nc.declare_dram_parameter("Q8", [EPC, D, D], F32, isOutput=False)
    q8 = nc.declare_dram_parameter("q8", [EPC, D], F32, isOutput=False)
    Ai8 = nc.declare_dram_parameter("Ai8", [EPC, MI, D], F32, isOutput=False)
    bi8 = nc.declare_dram_parameter("bi8", [EPC, MI], F32, isOutput=False)
    Ae8 = nc.declare_dram_parameter("Ae8", [EPC, ME, D], F32, isOutput=False)
    be8 = nc.declare_dram_parameter("be8", [EPC, ME], F32, isOutput=False)
    x8 = nc.declare_dram_parameter("x8", [EPC, D], F32, isOutput=False)
    identD = nc.declare_dram_parameter("identD", [128, 128], F32, isOutput=False)
    xs8 = nc.declare_dram_parameter("xs8", [EPC, D], F32, isOutput=True)

    # DRAM scratch: E1 (final solve) and G2 (clean rounds), reloaded later
    E1d = nc.dram_tensor("E1d", [EPC, 128, MC * D], BF16)
    G2d = nc.dram_tensor("G2d", [EPC, 128, MC * M], BF16)

    with tile.TileContext(nc) as tc:
        with tc.tile_pool(name="pers", bufs=1) as P0:
            ident = P0.tile([128, 128], F32)
            nc.sync.dma_start(ident[:], identD[:])
            identb = P0.tile([128, 128], BF16)
            nc.vector.tensor_copy(identb[:], ident[:])

            # persistent state (all [128, chunk, elem] layouts)
            G1t = P0.tile([128, EPC, MC, M], BF16)
            tcol = P0.tile([128, MC, EPC], F32)
            vcol = P0.tile([128, MC, EPC], F32)
            zcol = P0.tile([128, MC, EPC], F32)
            ccol = P0.tile([128, MC, EPC], F32)
            ecol = P0.tile([128, MC, EPC], F32)
            zetmp = P0.tile([128, MC, EPC], F32)
            sfc = P0.tile([128, MC, EPC], F32)
            uineq = P0.tile([128, 4, EPC], F32)
            s1c = P0.tile([128, MC, EPC], BF16)
            s2c = P0.tile([128, MC, EPC], BF16)
            trowAa = P0.tile([128, 384], BF16)
            trowAb = P0.tile([128, 256], BF16)
            trowBa = P0.tile([128, 384], BF16)
            trowBb = P0.tile([128, 256], BF16)
            y0colP = P0.tile([128, NC, EPC], F32)

            # ---------------- per-element precompute ----------------
            with (
                tc.tile_pool(name="pre", bufs=1) as P1,
                tc.tile_pool(name="prep", bufs=1, space="PSUM") as PSA,
            ):
                def split_chunk(dst1, dst2, src_f32):
                    """dst1/dst2 (bf16 APs) = hi/lo split of src_f32 AP."""
                    nc.scalar.copy(dst1, src_f32)
                    nc.vector.tensor_sub(dst2, src_f32, dst1)

                def split_chunk_g(dst1, dst2, src_f32):
                    """split with the lo-sub on gpsimd (SBUF sources only)."""
                    nc.scalar.copy(dst1, src_f32)
                    nc.gpsimd.tensor_sub(dst2, src_f32, dst1)

                for e in range(EPC):
                    # -- load A ([m-part, mchunk, d] layout) and split
                    A5f = P1.tile([128, MC, D], F32, tag="A5f")
                    nc.sync.dma_start(
                        A5f[:, 0:4, :],
                        Ai8[e].rearrange("(c p) d -> p c d", p=128))
                    nc.sync.dma_start(A5f[:, 4, :], Ae8[e])
                    A5b1 = P1.tile([128, MC, D], BF16, tag="A5b1", bufs=2)
                    A5b2 = P1.tile([128, MC, D], BF16, tag="A5b2")
                    for j in range(MC):
                        split_chunk_g(A5b1[:, j, :], A5b2[:, j, :], A5f[:, j, :])

                    # -- M = (A1+A2)^T A1 + Q + I  (f32 Mf kept; M1b bf16)
                    Mf = P1.tile([128, NC, D], F32, tag="Mf")
                    M1b = P1.tile([128, NC, D], BF16, tag="M1b", bufs=2)
                    for i in range(NC):
                        ps = PSA.tile([128, D], F32, tag="acc", bufs=2)
                        for pi, la in enumerate((A5b1, A5b2)):
                            for j in range(MC):
                                nc.tensor.matmul(
                                    ps[:], la[:, j, 128 * i:128 * (i + 1)],
                                    A5b1[:, j, :],
                                    start=(pi == 0 and j == 0),
                                    stop=(pi == 1 and j == MC - 1))
                        qblk = P1.tile([128, D], F32, tag="qblk", bufs=2)
                        nc.sync.dma_start(
                            qblk[:],
                            Q8[e].rearrange("(c p) d -> p c d", p=128)[:, i, :])
                        nc.vector.tensor_add(Mf[:, i, :], ps[:], qblk[:])
                        nc.vector.tensor_add(
                            Mf[:, i, 128 * i:128 * (i + 1)],
                            Mf[:, i, 128 * i:128 * (i + 1)], ident[:])
                        nc.vector.tensor_copy(M1b[:, i, :], Mf[:, i, :])

                    # -- M2 = M1 @ M1 ; Q2b = bf16(P3 Mf + P4 M2)
                    M2f = P1.tile([128, NC, D], F32, tag="M2f")
                    M2b = P1.tile([128, NC, D], BF16, tag="M2b")
                    Q2b = P1.tile([128, NC, D], BF16, tag="Q2b")
                    for i in range(NC):
                        ps = PSA.tile([128, D], F32, tag="acc", bufs=2)
                        for k in range(NC):
                            nc.tensor.matmul(
                                ps[:], M1b[:, k, 128 * i:128 * (i + 1)],
                                M1b[:, k, :],
                                start=(k == 0), stop=(k == NC - 1))
                        nc.vector.tensor_copy(M2f[:, i, :], ps[:])
                        t3 = P1.tile([128, D], F32, tag="t3", bufs=2)
                        nc.vector.tensor_scalar_mul(t3[:], Mf[:, i, :], P3c)
                        nc.vector.scalar_tensor_tensor(
                            Q2b[:, i, :], ps[:], P4c, t3[:],
                            op0=ALU.mult, op1=ALU.add)
                        nc.scalar.copy(M2b[:, i, :], ps[:])

                    # -- X0 = P0 I + P1 Mf + P2 M2f + M2b @ Q2b
                    X0f = P1.tile([128, NC, D], F32, tag="X0f")
                    X1p = P1.tile([128, NC, D], BF16, tag="X1p")
                    for i in range(NC):
                        ps = PSA.tile([128, D], F32, tag="acc", bufs=2)
                        for k in range(NC):
                            nc.tensor.matmul(
                                ps[:], M2b[:, k, 128 * i:128 * (i + 1)],
                                Q2b[:, k, :],
                                start=(k == 0), stop=(k == NC - 1))
                        nc.vector.scalar_tensor_tensor(
                            X0f[:, i, :], Mf[:, i, :], P1c, ps[:],
                            op0=ALU.mult, op1=ALU.add)
                        nc.vector.scalar_tensor_tensor(
                            X0f[:, i, :], M2f[:, i, :], P2c, X0f[:, i, :],
                            op0=ALU.mult, op1=ALU.add)
                        nc.vector.scalar_tensor_tensor(
                            X0f[:, i, 128 * i:128 * (i + 1)], ident[:], P0c,
                            X0f[:, i, 128 * i:128 * (i + 1)],
                            op0=ALU.mult, op1=ALU.add)
                        nc.vector.tensor_copy(X1p[:, i, :], X0f[:, i, :])

                    # -- fp32-split Newton polish: R = I - M1(X1p + X2p),
                    #    X = X0 + X1p R
                    Rm = P1.tile([128, NC, D], BF16, tag="Rm")
                    for i in range(NC):
                        ps = PSA.tile([128, D], F32, tag="acc", bufs=2)
                        for k in range(NC):
                            nc.tensor.matmul(
                                ps[:], M1b[:, k, 128 * i:128 * (i + 1)],
                                X1p[:, k, :],
                                start=(k == 0), stop=(k == NC - 1))
                        nc.vector.tensor_scalar_mul(Rm[:, i, :], ps[:], -1.0)
                        rfd = P1.tile([128, 128], F32, tag="rfd", bufs=1)
                        nc.vector.tensor_sub(
                            rfd[:], ident[:], ps[:, 128 * i:128 * (i + 1)])
                        nc.vector.tensor_copy(
                            Rm[:, i, 128 * i:128 * (i + 1)], rfd[:])
                    Xf = P1.tile([128, NC, D], F32, tag="Xf")
                    X1 = P1.tile([128, NC, D], BF16, tag="X1", bufs=2)
                    for i in range(NC):
                        ps = PSA.tile([128, D], F32, tag="acc", bufs=2)
                        for k in range(NC):
                            nc.tensor.matmul(
                                ps[:], X1p[:, k, 128 * i:128 * (i + 1)],
                                Rm[:, k, :],
                                start=(k == 0), stop=(k == NC - 1))
                        nc.vector.tensor_add(Xf[:, i, :], X0f[:, i, :], ps[:])
                        nc.vector.tensor_copy(X1[:, i, :], Xf[:, i, :])

                    # -- AT1 = A5b1^T ([d-part, dchunk, m]) via PE transposes
                    AT1 = P1.tile([128, NC, M], BF16, tag="AT1", bufs=2)
                    for j in range(MC):
                        for k in range(NC):
                            tp = PSA.tile([128, 128], BF16, tag="tp", bufs=2)
                            nc.tensor.transpose(
                                tp[:], A5b1[:, j, 128 * k:128 * (k + 1)],
                                identb[:])
                            if k % 2 == 0:
                                nc.vector.tensor_copy(
                                    AT1[:, k, 128 * j:128 * (j + 1)], tp[:])
                            else:
                                nc.scalar.copy(
                                    AT1[:, k, 128 * j:128 * (j + 1)], tp[:])

                    # -- q2 = q - x (column form [128, NC]) and splits
                    qc = P1.tile([128, NC], F32, tag="qc", bufs=2)
                    xc = P1.tile([128, NC], F32, tag="xc", bufs=2)
                    nc.sync.dma_start(qc[:], q8[e].rearrange("(c p) -> p c", p=128))
                    nc.sync.dma_start(xc[:], x8[e].rearrange("(c p) -> p c", p=128))
                    q2c = P1.tile([128, NC], F32, tag="q2c", bufs=2)
                    nc.vector.tensor_sub(q2c[:], qc[:], xc[:])
                    q21 = P1.tile([128, NC], BF16, tag="q21", bufs=2)
                    q22 = P1.tile([128, NC], BF16, tag="q22", bufs=2)
                    split_chunk_g(q21[:], q22[:], q2c[:])

                    # -- clip bounds: ineq into uineq, eq into zcol chunk 4
                    nc.sync.dma_start(
                        uineq[:, :, e], bi8[e].rearrange("(c p) -> p c", p=128))
                    nc.sync.dma_start(zcol[:, 4, e:e + 1],
                                      be8[e:e + 1].rearrange('o p -> p o'))

                    # -- y0 row = (q21+q22)^T X1, then column
                    psr = PSA.tile([1, D], F32, tag="row", bufs=1)
                    for pi, la in enumerate((q21, q22)):
                        for k in range(NC):
                            nc.tensor.matmul(
                                psr[:], la[:, k:k + 1], X1[:, k, :],
                                start=(pi == 0 and k == 0),
                                stop=(pi == 1 and k == NC - 1))
                    y0rt = P1.tile([1, M], F32, tag="rowst", bufs=1)
                    nc.vector.tensor_copy(y0rt[:, 0:D], psr[:])
                    for k in range(NC):
                        tp = PSA.tile([128, 128], F32, tag="tpf", bufs=1)
                        nc.tensor.transpose(
                            tp[:, 0:1],
                            y0rt[:, 128 * k:128 * (k + 1)],
                            ident[0:1, 0:1])
                        nc.vector.tensor_copy(y0colP[:, k, e:e + 1], tp[:, 0:1])
                    y01 = P1.tile([128, NC], BF16, tag="y01", bufs=2)
                    y02 = P1.tile([128, NC], BF16, tag="y02", bufs=2)
                    split_chunk_g(y01[:], y02[:], y0colP[:, :, e])

                    # -- e row = (y01+y02)^T AT1 (spans 384 + 256) -> ecol
                    erow = P1.tile([1, M], F32, tag="rowst", bufs=1)
                    for lo, hi in ((0, 384), (384, 640)):
                        pse = PSA.tile([1, D], F32, tag="row", bufs=1)
                        for pi, la in enumerate((y01, y02)):
                            for k in range(NC):
                                nc.tensor.matmul(
                                    pse[:, 0:hi - lo], la[:, k:k + 1],
                                    AT1[:, k, lo:hi],
                                    start=(pi == 0 and k == 0),
                                    stop=(pi == 1 and k == NC - 1))
                        nc.vector.tensor_copy(erow[:, lo:hi], pse[:, 0:hi - lo])
                    for j in range(MC):
                        tp = PSA.tile([128, 128], F32, tag="tpf", bufs=1)
                        nc.tensor.transpose(
                            tp[:, 0:1], erow[:, 128 * j:128 * (j + 1)],
                            ident[0:1, 0:1])
                        nc.vector.tensor_copy(ecol[:, j, e:e + 1], tp[:, 0:1])

                    # -- Dm = X1 @ AT1 ([d-part, dchunk, m]), D1 bf16 (1-pass)
                    D1 = P1.tile([128, NC, M], BF16, tag="D1", bufs=2)
                    for i in range(NC):
                        for lo, hi in ((0, 384), (384, 640)):
                            ps = PSA.tile([128, 384], F32, tag="accm", bufs=2)
                            for k in range(NC):
                                nc.tensor.matmul(
                                    ps[:, 0:hi - lo],
                                    X1[:, k, 128 * i:128 * (i + 1)],
                                    AT1[:, k, lo:hi],
                                    start=(k == 0), stop=(k == NC - 1))
                            if lo == 0:
                                nc.scalar.copy(D1[:, i, lo:hi], ps[:, 0:hi - lo])
                            else:
                                nc.vector.tensor_copy(
                                    D1[:, i, lo:hi], ps[:, 0:hi - lo])

                    # -- E1 = D1^T via bf16 PE transposes, spilled to DRAM
                    E1 = P1.tile([128, MC, D], BF16, tag="E1", bufs=2)
                    for j in range(MC):
                        for k in range(NC):
                            tp = PSA.tile([128, 128], BF16, tag="tp", bufs=2)
                            nc.tensor.transpose(
                                tp[:], D1[:, k, 128 * j:128 * (j + 1)],
                                identb[:])
                            if k % 2 == 0:
                                nc.vector.tensor_copy(
                                    E1[:, j, 128 * k:128 * (k + 1)], tp[:])
                            else:
                                nc.scalar.copy(
                                    E1[:, j, 128 * k:128 * (k + 1)], tp[:])
                    nc.sync.dma_start(E1d[e], E1[:].rearrange("p c d -> p (c d)"))

                    # -- G = AT1^T @ D1, split G1 (SBUF) / G2 (DRAM spill)
                    G2e = P1.tile([128, MC, M], BF16, tag="G2e", bufs=1)
                    for j in range(MC):
                        for lo, hi in ((0, 384), (384, 640)):
                            ps = PSA.tile([128, 384], F32, tag="accm", bufs=2)
                            for k in range(NC):
                                nc.tensor.matmul(
                                    ps[:, 0:hi - lo],
                                    AT1[:, k, 128 * j:128 * (j + 1)],
                                    D1[:, k, lo:hi],
                                    start=(k == 0), stop=(k == NC - 1))
                            split_chunk(G1t[:, e, j, lo:hi],
                                        G2e[:, j, lo:hi], ps[:, 0:hi - lo])
                    nc.sync.dma_start(G2d[e], G2e[:].rearrange("p c d -> p (c d)"))

            # ---------------- ADMM iterations ----------------
            def tail(w, boot=False, with_s2=True):
                S = slice(4 * w, 4 * w + 4)
                if boot:
                    nc.vector.tensor_scalar_mul(
                        vcol[:, :, S], ecol[:, :, S], -1.0)
                else:
                    nc.vector.scalar_tensor_tensor(
                        vcol[:, :, S], tcol[:, :, S], ALPHA, ccol[:, :, S],
                        op0=ALU.mult, op1=ALU.add)
                nc.vector.tensor_tensor(
                    zcol[:, 0:4, S], vcol[:, 0:4, S], uineq[:, :, S],
                    op=ALU.min)
                nc.vector.scalar_tensor_tensor(
                    s1c[:, :, S], zcol[:, :, S], 2.0, vcol[:, :, S],
                    op0=ALU.mult, op1=ALU.subtract)
                if with_s2:
                    nc.vector.scalar_tensor_tensor(
                        sfc[:, :, S], zcol[:, :, S], 2.0, vcol[:, :, S],
                        op0=ALU.mult, op1=ALU.subtract)
                    nc.gpsimd.tensor_sub(
                        s2c[:, :, S], sfc[:, :, S], s1c[:, :, S])
                nc.gpsimd.tensor_add(
                    zetmp[:, :, S], zcol[:, :, S], ecol[:, :, S])
                nc.vector.scalar_tensor_tensor(
                    ccol[:, :, S], zetmp[:, :, S], -ALPHA, vcol[:, :, S],
                    op0=ALU.mult, op1=ALU.add)

            with tc.tile_pool(name="adm", bufs=1) as PA:
              with tc.tile_pool(name="itp", bufs=1, space="PSUM") as PSI:
                G2sb = PA.tile([128, EPC, MC, M], BF16)
                for e in range(EPC):
                    nc.sync.dma_start(
                        G2sb[:, e].rearrange("p c d -> p (c d)"), G2d[e])
                E1all = PA.tile([128, EPC, MC, D], BF16)
                for e in range(EPC):
                    nc.sync.dma_start(
                        E1all[:, e].rearrange("p c d -> p (c d)"), E1d[e])

                tail(0, boot=True)
                tail(1, boot=True)

                def mm_wave(wave, k, clean):
                    pA = PSI.tile([128, 384], F32, tag="wvA", bufs=2,
                                  name=f"pA_{k}_{wave}")
                    pB = PSI.tile([128, 256], F32, tag="wvB", bufs=2,
                                  name=f"pB_{k}_{wave}")
                    passes = (((G1t, s1c), (G2sb, s1c), (G1t, s2c))
                              if clean else ((G1t, s1c),))
                    np_ = len(passes)
                    for pi, (Gt, st) in enumerate(passes):
                        for j in range(MC):
                            first = pi == 0 and j == 0
                            last = pi == np_ - 1 and j == MC - 1
                            for eo in range(4):
                                e = 4 * wave + eo
                                ga = (Gt[:, e, j, 0:384] if Gt is G1t
                                      else Gt[:, e, j, 0:384])
                                gb = (Gt[:, e, j, 384:640] if Gt is G1t
                                      else Gt[:, e, j, 384:640])
                                nc.tensor.matmul(
                                    pA[32 * eo:32 * eo + 1, :],
                                    st[:, j, e:e + 1], ga,
                                    start=first, stop=last,
                                    tile_position=(0, 32 * eo))
                                nc.tensor.matmul(
                                    pB[32 * eo:32 * eo + 1, :],
                                    st[:, j, e:e + 1], gb,
                                    start=first, stop=last,
                                    tile_position=(0, 32 * eo))
                    return pA, pB

                def post_wave(wave, pA, pB, k, with_s2=True):
                    ta = trowAa if wave == 0 else trowBa
                    tb = trowAb if wave == 0 else trowBb
                    # full-tile copies: rows 32*eo carry t, the rest is
                    # garbage the column extraction never reads; 128 lanes
                    # make these ~8x cheaper than per-row copies
                    nc.vector.tensor_copy(ta[:], pA[:])
                    nc.scalar.copy(tb[:], pB[:])
                    T2a = PSI.tile([128, 3, 128], BF16, tag="T2a", bufs=2,
                                   name=f"t2a_{k}_{wave}")
                    T2b = PSI.tile([128, 2, 128], BF16, tag="T2b", bufs=2,
                                   name=f"t2b_{k}_{wave}")
                    for j in range(MC):
                        src_ap = (ta[:, 128 * j:128 * (j + 1)] if j < 3
                                  else tb[:, 128 * (j - 3):128 * (j - 2)])
                        dst = T2a[:, j, :] if j < 3 else T2b[:, j - 3, :]
                        nc.tensor.transpose(dst, src_ap, identb[:])
                    S4 = slice(4 * wave, 4 * wave + 4)
                    nc.vector.tensor_copy(
                        tcol[:, 0:3, S4],
                        T2a.rearrange("p c (a b) -> p c a b", b=32)[:, :, :, 0])
                    nc.scalar.copy(
                        tcol[:, 3:5, S4],
                        T2b.rearrange("p c (a b) -> p c a b", b=32)[:, :, :, 0])
                    tail(wave, with_s2=with_s2)

                for k in range(N_FAST + N_CLEAN):
                    clean = k >= N_FAST
                    with_s2 = k >= N_FAST - 1
                    pA0, pB0 = mm_wave(0, k, clean)
                    pA1, pB1 = mm_wave(1, k, clean)
                    post_wave(0, pA0, pB0, k, with_s2=with_s2)
                    post_wave(1, pA1, pB1, k, with_s2=with_s2)

              # ------------- final solve: xs = E1^T (s1+s2) - y0 ----------
              with (
                tc.tile_pool(name="fin", bufs=1) as PF,
                tc.tile_pool(name="finp", bufs=1, space="PSUM") as PSF,
              ):
                for g in range(2):
                    ps4 = PSF.tile([128, D], F32, tag="fr4", bufs=2)
                    for pi, st in enumerate((s1c, s2c)):
                        for j in range(MC):
                            first = pi == 0 and j == 0
                            last = pi == 1 and j == MC - 1
                            for eo in range(4):
                                e = 4 * g + eo
                                nc.tensor.matmul(
                                    ps4[32 * eo:32 * eo + 1, :],
                                    st[:, j, e:e + 1], E1all[:, e, j, :],
                                    start=first, stop=last,
                                    tile_position=(0, 32 * eo))
                    # rows -> columns (reuse the ADMM transpose path), then
                    # subtract y0 in column space and DMA out column-major
                    xrow = PF.tile([128, D], F32, tag="xrow", bufs=2)
                    nc.vector.tensor_copy(xrow[:], ps4[:])
                    xcol = PF.tile([128, NC, 4], F32, tag="xcol", bufs=2)
                    for kk in range(NC):
                        tpf = PSF.tile([128, 128], F32, tag="tpf", bufs=2)
                        nc.tensor.transpose(
                            tpf[:], xrow[:, 128 * kk:128 * (kk + 1)],
                            ident[:])
                        tps = tpf.rearrange("p (a b) -> p a b", b=32)
                        nc.vector.tensor_copy(xcol[:, kk, :], tps[:, :, 0])
                    nc.vector.tensor_sub(
                        xcol[:], xcol[:], y0colP[:, :, 4 * g:4 * g + 4])
                    for eo in range(4):
                        e = 4 * g + eo
                        nc.sync.dma_start(
                            xs8[e].rearrange("(c p) -> p c", p=128),
                            xcol[:, :, eo])

    nc.finalize()
    return nc


_CACHED = {}


def _get_program():
    if "nc" not in _CACHED:
        _CACHED["nc"] = build_program()
    return _CACHED["nc"]


def run(inputs, trace=False, trace_cores=None):
    nc = _get_program()
    Q = np.ascontiguousarray(inputs["Q"], dtype=np.float32)
    q = np.ascontiguousarray(inputs["q"], dtype=np.float32)[..., 0]
    Ai = np.ascontiguousarray(inputs["A_ineq"], dtype=np.float32)
    bi = np.ascontiguousarray(inputs["b_ineq"], dtype=np.float32)[..., 0]
    Ae = np.ascontiguousarray(inputs["A_eq"], dtype=np.float32)
    be = np.ascontiguousarray(inputs["b_eq"], dtype=np.float32)[..., 0]
    x = np.ascontiguousarray(inputs["x"], dtype=np.float32)[..., 0]
    ident = np.eye(128, dtype=np.float32)

    in_maps = []
    for c in range(8):
        s = slice(EPC * c, EPC * (c + 1))
        in_maps.append({
            "Q8": Q[s], "q8": q[s], "Ai8": Ai[s], "bi8": bi[s],
            "Ae8": Ae[s], "be8": be[s], "x8": x[s], "identD": ident,
        })
    res = bass_utils.run_bass_kernel_spmd(
        nc, in_maps, list(range(8)), trace=trace,
        trace_cores=trace_cores)
    out = np.concatenate([res.results[c]["xs8"] for c in range(8)], axis=0)
    return out[..., None].astype(np.float32), res


def kernel(**inputs):
    out, _ = run(inputs, trace=False)
    return out
